# revision 25
# baseline (speedup 1.0000x reference)
"""Trainium2 Bass kernel for nn_AttnConv2d (attention-conv + dynamic conv + BN).

Math (per sample b):
  a1 = conv3x3(x, w1); a2 = conv3x3(x, w2); a3 = conv3x3(x, w3)     (SAME pad)
  attn[h,w,i,o] = sum_{p,q} a1[i,3p+h,3q+w] * a2[o,3p+h,3q+w]
  kern[o,:,:,:] = softmax(attn[.,.,.,o] / sqrt(Ci*9))
  av = conv3x3(a3, kern[b])                                         (per-sample kernel)
  y  = feature_map_stack(av)   (pure spatial/channel permutation)
  out = cm * x + NORM_SCALE * (y - mean_y) * rsqrt(var_y + eps)     (batch stats)

Sharding: data-parallel over batch, 1 sample per core, 8 cores.  The only
cross-core exchange is an AllReduce of the per-channel BN partial sums.

Implementation notes:
  - x arrives host-padded ([128, H+2, W+2]) in bf16 (a1/a2 convs) and fp8
    (a3 conv), so convs are 9 shifted accumulating matmuls into PSUM with
    no on-device edge handling.  a3 and the dynamic conv run fp8 with
    DoubleRow perf mode (two kernel offsets per matmul, K=256); fp8 there
    costs ~1e-2 rel err (attention path must stay bf16: fp8 a1/a2 alone
    is 1.8e-2).
  - attention contraction needs positions on the partition axis: conv
    outputs are scatter-copied to subgrid-major SBUF (bf16), PE-transposed
    in 128-position chunks, then accumulated into a persistent PSUM tile.
  - a2's output-channel order is permuted host-side (partition p holds
    channel 4*(p%32) + p//32) so feature_map_stack becomes a
    per-(partition,parity) affine map on av.
  - feature_map_stack is applied strip-by-strip during pass B as
    SBUF->SBUF scatter DMAs into a y-layout tile, so the permutation's
    descriptor cost overlaps the dynamic-conv compute; pass C then runs
    entirely on clean contiguous DMAs (x strip load, out strip store).
  - BN group-of-4 partition sums are a tiny 0/1 matmul; x prefetch is
    issued before the AllReduce so the collective latency is hidden.
  - strip pipeline is skewed: transposes+attention for strip s issue after
    the convs of strip s+1, so the scalar-engine scatter never stalls PE.
"""

import os
import sys

for _p in ("/opt/trn_rl_repo", "/root/.axon_site/_ro/trn_rl_repo"):
    if os.path.isdir(_p) and _p not in sys.path:
        sys.path.insert(0, _p)
        break

import numpy as np

import concourse.bass as bass
import concourse.bacc as bacc
import concourse.tile as tile
from concourse import mybir

F32 = mybir.dt.float32
BF16 = mybir.dt.bfloat16
FP8 = mybir.dt.float8e4
DR = mybir.MatmulPerfMode.DoubleRow

EPS = 1e-5
NORM_SCALE = 0.1816
CI = 128

# a3/dynamic-conv path runs fp8 (attention path stays bf16); DR_CONV
# selects DoubleRow pairing (2 offsets per matmul) vs plain fp8 matmuls.
DR_CONV = os.environ.get("DR_CONV", "1") == "1"


def _rap(base, dims, off=0):
    """Raw AP on the same tensor as `base` (keeps base's partition dim)."""
    return bass.AP(tensor=base.tensor, offset=base.offset + off,
                   ap=[base.ap[0]] + [list(d) for d in dims])


def _conv_tile(nc, cps, wt, xs, base, W, XW, dr):
    """Accumulate the 9-offset conv into PSUM cps [128, 2*W] for the row
    pair whose top-left (unshifted) element is at linear offset `base`
    within xs (a padded [128, rows, XW] tile)."""
    if not dr:
        for k in range(9):
            dy, dx = divmod(k, 3)
            rhs = _rap(xs[:], [[XW, 2], [1, W]], base + dy * XW + dx)
            nc.tensor.matmul(cps[:, :], wt[:, k, :], rhs,
                             start=(k == 0), stop=(k == 8))
        return
    deltas = [dy * XW + dx for dy in range(3) for dx in range(3)]
    for j in range(4):
        da, db = deltas[2 * j], deltas[2 * j + 1]
        rhs = _rap(xs[:], [[db - da, 2], [XW, 2], [1, W]], base + da)
        nc.tensor.matmul(cps[:, :], wt[:, 2 * j:2 * j + 2, :], rhs,
                         start=(j == 0), stop=False, perf_mode=DR,
                         skip_group_check=True)
    rhs = _rap(xs[:], [[XW, 2], [1, W]], base + deltas[8])
    nc.tensor.matmul(cps[:, :], wt[:, 8, :], rhs,
                     start=False, stop=True, skip_group_check=True)


def build_nc(H, W, R, n_cores, cm, level=5):
    """Build the per-core Bass kernel. R = strip rows (div by 6, even)."""
    assert H % R == 0 and R % 6 == 0 and W % 6 == 0
    NS = H // R                      # strips
    Wq = W // 3                      # attn subgrid cols
    P = (R // 3) * Wq                # attn positions per offset per strip
    S = H // 2                       # quadrant size of feature_map_stack
    NT = R // 2                      # psum tiles (2 rows) per strip
    SR = R // 2                      # subgrid rows per strip (parity space)
    SQ = W // 2                      # subgrid cols (parity space)
    N_TOT = float(n_cores * H * W)   # BN count per channel
    SCL = 1.0 / float(np.sqrt(CI * 9))
    XW = W + 2                       # padded row pitch

    nc = bacc.Bacc("TRN2", target_bir_lowering=False, debug=False,
                   num_devices=n_cores)

    xb_in = nc.dram_tensor("xb", [128, H + 2, XW], BF16,
                           kind="ExternalInput").ap()
    x8_in = nc.dram_tensor("x8", [128, H + 2, XW], FP8,
                           kind="ExternalInput").ap()
    w1_in = nc.dram_tensor("w1t", [128, 9, 128], BF16,
                           kind="ExternalInput").ap()
    w2_in = nc.dram_tensor("w2t", [128, 9, 128], BF16,
                           kind="ExternalInput").ap()
    w3_in = nc.dram_tensor("w3t", [128, 9, 128], FP8,
                           kind="ExternalInput").ap()
    id_in = nc.dram_tensor("ident", [128, 128], BF16, kind="ExternalInput").ap()
    gp_in = nc.dram_tensor("gsum", [128, 128], F32, kind="ExternalInput").ap()
    mk_in = nc.dram_tensor("mask4", [128, 4], F32, kind="ExternalInput").ap()
    out_d = nc.dram_tensor("out", [128, H, W], F32, kind="ExternalOutput").ap()
    avp_d = nc.dram_tensor("avp", [128, H, W], FP8).ap()   # scratch, y layout

    with tile.TileContext(nc) as tc:
        consts = tc.alloc_tile_pool(name="consts", bufs=1)
        w1t = consts.tile([128, 9, 128], BF16, tag="w1t")
        w2t = consts.tile([128, 9, 128], BF16, tag="w2t")
        w3t = consts.tile([128, 9, 128], FP8, tag="w3t")
        ident = consts.tile([128, 128], BF16, tag="ident")
        gsum = consts.tile([128, 128], F32, tag="gsum")
        mask4 = consts.tile([128, 4], F32, tag="mask4")
        nc.sync.dma_start(out=w1t[:], in_=w1_in[:])
        nc.sync.dma_start(out=w2t[:], in_=w2_in[:])
        nc.sync.dma_start(out=w3t[:], in_=w3_in[:])
        nc.sync.dma_start(out=ident[:], in_=id_in[:])
        nc.sync.dma_start(out=gsum[:], in_=gp_in[:])
        nc.sync.dma_start(out=mask4[:], in_=mk_in[:])

        small = tc.alloc_tile_pool(name="small", bufs=1)
        stats_cols = small.tile([128, NS, 4, 2], F32, tag="stats_cols")
        sloc = small.tile([128, 8], F32, tag="sloc")
        sglob = small.tile([128, 8], F32, tag="sglob")
        scalars = small.tile([128, 16], F32, tag="scalars")
        msb = small.tile([128, 8], F32, tag="msb")
        sel = small.tile([128, 4], F32, tag="sel")

        # xb strips persist from pass A through pass C (residual input)
        pa_xb = tc.alloc_tile_pool(name="pa_xb", bufs=NS)
        # pass-C av strip tiles (prefetched during pass B)
        pc_a = tc.alloc_tile_pool(name="pc_a", bufs=8)

        kern_pool = tc.alloc_tile_pool(name="kern", bufs=1)
        kern8 = kern_pool.tile([128, 9, 128], FP8, tag="kern8")

        a3_pool = tc.alloc_tile_pool(name="a3p", bufs=1)
        a3p = a3_pool.tile([128, H + 2, XW], FP8, tag="a3p")
        # zero the pad border of a3p once
        nc.vector.memset(_rap(a3p[:], [[1, XW]]), 0.0)                    # row 0
        nc.vector.memset(_rap(a3p[:], [[1, XW]], (H + 1) * XW), 0.0)      # row H+1
        nc.vector.memset(_rap(a3p[:], [[XW, H + 2]]), 0.0)                # col 0
        nc.vector.memset(_rap(a3p[:], [[XW, H + 2]], W + 1), 0.0)         # col W+1

        attn_psp = tc.alloc_tile_pool(name="attn_ps", bufs=1, space="PSUM")
        attn_ps = attn_psp.tile([128, 9 * 128], F32, tag="attn")

        # ---------------- pass A: static convs + attention accumulation ------
        pa_x = tc.alloc_tile_pool(name="pa_x", bufs=2)
        pa_g = tc.alloc_tile_pool(name="pa_g", bufs=2)
        pa_t = tc.alloc_tile_pool(name="pa_t", bufs=2)
        pa_cps = tc.alloc_tile_pool(name="pa_cps", bufs=3, space="PSUM")
        pa_tps = tc.alloc_tile_pool(name="pa_tps", bufs=2, space="PSUM")

        a1gs = {}
        a2gs = {}
        xsbs = {}

        def conv_strip(s):
            y0 = s * R
            xsb = pa_xb.tile([128, R + 2, XW], BF16, tag="xsb")
            xsbs[s] = xsb
            nc.sync.dma_start(out=xsb[:], in_=xb_in[:, y0:y0 + R + 2, :])
            xs8 = pa_x.tile([128, R + 2, XW], FP8, tag="xs8")
            nc.sync.dma_start(out=xs8[:], in_=x8_in[:, y0:y0 + R + 2, :])
            a1g = pa_g.tile([128, 9, P], BF16, tag="a1g")
            a2g = pa_g.tile([128, 9, P], BF16, tag="a2g")
            a1gs[s] = a1g
            a2gs[s] = a2g
            for wt, gdst, xt, fp8 in ((w1t, a1g, xsb, False),
                                      (w2t, a2g, xsb, False),
                                      (w3t, None, xs8, DR_CONV)):
                for t in range(NT):
                    cps = pa_cps.tile([128, 2 * W], F32, tag="cps")
                    _conv_tile(nc, cps, wt, xt, 2 * t * XW, W, XW, fp8)
                    if gdst is not None:
                        # scatter rows (2t, 2t+1) into subgrid-major layout
                        ya, yb = 2 * t, 2 * t + 1
                        ha, ra = ya % 3, ya // 3
                        hb, rb = yb % 3, yb // 3
                        offa = (3 * ha) * P + ra * Wq
                        sd = (3 * hb) * P + rb * Wq - offa
                        nc.scalar.copy(
                            out=_rap(gdst[:], [[sd, 2], [P, 3], [1, Wq]],
                                     offa),
                            in_=_rap(cps[:], [[W, 2], [1, 3], [3, Wq]]))
                    else:
                        nc.scalar.copy(
                            out=a3p[:, 1 + y0 + 2 * t:1 + y0 + 2 * t + 2,
                                    1:1 + W],
                            in_=_rap(cps[:], [[W, 2], [1, W]]))

        def attn_strip(s):
            a1g, a2g = a1gs.pop(s), a2gs.pop(s)
            for hw in range(9):
                for c0 in range(0, P, 256):
                    t4 = pa_tps.tile([128, 4, 128], BF16, tag="tps")
                    for j, (g, cc) in enumerate(((a1g, c0), (a2g, c0),
                                                 (a1g, c0 + 128),
                                                 (a2g, c0 + 128))):
                        nc.tensor.transpose(t4[:, j, :], g[:, hw, cc:cc + 128],
                                            ident[:])
                    aT4 = pa_t.tile([128, 4, 128], BF16, tag="aT")
                    nc.vector.tensor_copy(aT4[:], t4[:])
                    for j in range(2):
                        nc.tensor.matmul(
                            attn_ps[:, hw * 128:(hw + 1) * 128],
                            aT4[:, 2 * j + 1, :], aT4[:, 2 * j, :],
                            start=(s == 0 and c0 == 0 and j == 0
                                   and hw in (0, 4, 8)),
                            stop=(s == NS - 1 and c0 + 256 >= P and j == 1
                                  and hw in (3, 7, 8)),
                            skip_group_check=True)

        # skewed pipeline: the scalar/vector scatter of strip s drains while
        # PE runs the convs of strip s+1.
        for s in range(NS):
            conv_strip(s)
            if s > 0:
                attn_strip(s - 1)
        attn_strip(NS - 1)

        pa_tps.release(); pa_cps.release()
        pa_t.release(); pa_g.release(); pa_x.release()
        # pa_xb stays live: pass C reuses the xb strips for the residual

        # ---------------- softmax + kern transposes -------------------------
        if level >= 2:
            sm_pool = tc.alloc_tile_pool(name="smx", bufs=1)
            attn_sb = sm_pool.tile([128, 9 * 128], F32, tag="attn_sb")
            nc.vector.tensor_copy(attn_sb[:], attn_ps[:])
            attn_psp.release()
            k_tps = tc.alloc_tile_pool(name="k_tps", bufs=2, space="PSUM")
            mx = scalars[:, 0:1]
            nmx = scalars[:, 1:2]
            ssum = scalars[:, 2:3]
            rsum = scalars[:, 3:4]
            nc.vector.reduce_max(mx, attn_sb[:], axis=mybir.AxisListType.X)
            nc.vector.tensor_scalar_mul(nmx, mx, -SCL)
            esb = sm_pool.tile([128, 9 * 128], F32, tag="esb")
            nc.scalar.activation(esb[:], attn_sb[:],
                                 mybir.ActivationFunctionType.Exp,
                                 bias=nmx, scale=SCL)
            nc.vector.reduce_sum(ssum, esb[:], axis=mybir.AxisListType.X)
            nc.vector.reciprocal(rsum, ssum)
            sm_bf = sm_pool.tile([128, 9 * 128], BF16, tag="sm_bf")
            nc.vector.tensor_scalar_mul(sm_bf[:], esb[:], rsum)
            for hw in range(9):
                tp = k_tps.tile([128, 128], BF16, tag="ktp")
                nc.tensor.transpose(tp[:], sm_bf[:, hw * 128:(hw + 1) * 128],
                                    ident[:])
                nc.vector.tensor_copy(kern8[:, hw, :], tp[:])
            k_tps.release(); sm_pool.release()
        else:
            attn_psp.release()

        # ---------------- pass B: dynamic conv + stats + y scatter ----------
        if level >= 3:
            pb_av = tc.alloc_tile_pool(name="pb_av", bufs=4)
            pb_sq = tc.alloc_tile_pool(name="pb_sq", bufs=2)
            pb_cps = tc.alloc_tile_pool(name="pb_cps", bufs=5, space="PSUM")
            avs = {}

            def dyn_strip(s):
                y0 = s * R
                # av parity-split strip: av_sp[c, 2i+j, p, q] = av[c, 2p+i, 2q+j]
                av_sp = pb_av.tile([128, 4, SR, SQ], FP8, tag="av_sp")
                avs[s] = av_sp
                for t in range(NT):
                    cps = pb_cps.tile([128, 2 * W], F32, tag="cps2")
                    _conv_tile(nc, cps, kern8, a3p, (y0 + 2 * t) * XW, W, XW,
                               DR_CONV)
                    nc.scalar.copy(
                        out=_rap(av_sp[:], [[2 * SR * SQ, 2], [SR * SQ, 2],
                                            [1, SQ]], t * SQ),
                        in_=_rap(cps[:], [[W, 2], [1, 2], [2, SQ]]))

            def stats_strip(s):
                av_sp = avs.pop(s)
                sq = pb_sq.tile([128, SR * SQ], BF16, tag="sq")
                for pi in range(4):
                    psrc = _rap(av_sp[:], [[1, SR * SQ]], pi * SR * SQ)
                    nc.vector.reduce_sum(stats_cols[:, s, pi, 0:1], psrc,
                                         axis=mybir.AxisListType.X)
                    nc.vector.scalar_tensor_tensor(
                        out=sq[:], in0=psrc, scalar=1.0, in1=psrc,
                        op0=mybir.AluOpType.mult, op1=mybir.AluOpType.mult,
                        accum_out=stats_cols[:, s, pi, 1:2])
                # feature_map_stack fold: av[32*c2+c1, par, p, q]
                #   -> avp[4*c1+par, S*(c2>>1)+s*SR+p, S*(c2&1)+q]
                for c2 in range(4):
                    qsrc = _rap(av_sp[32 * c2:32 * (c2 + 1)],
                                [[SR * SQ, 4], [SQ, SR], [1, SQ]])
                    dst = bass.AP(
                        tensor=avp_d.tensor,
                        offset=((c2 >> 1) * S * W + (c2 & 1) * SQ
                                + (s * SR) * W),
                        ap=[[4 * H * W, 32], [H * W, 4], [W, SR], [1, SQ]])
                    nc.gpsimd.dma_start(out=dst, in_=qsrc)

            avls = {}
            nxt = [0]

            def prefetch_ready(smax):
                # avp out-strip st (rows st*R..) is fully written once av
                # strips {2*st, 2*st+1} (st < NS//2: upper half) or
                # {2*st-NS, 2*st-NS+1} (lower half) have scattered.
                while nxt[0] < NS:
                    st = nxt[0]
                    need = 2 * st + 1 if st < NS // 2 else 2 * st - NS + 1
                    if need > smax:
                        break
                    avls[st] = _av_load(nc, pc_a, avp_d, st, R, W)
                    nxt[0] += 1

            for s in range(NS):
                dyn_strip(s)
                if s > 0:
                    stats_strip(s - 1)
                    prefetch_ready(s - 1)
            stats_strip(NS - 1)
            prefetch_ready(NS - 1)
            pb_cps.release()
            pb_sq.release()
            pb_av.release()
        a3_pool.release(); kern_pool.release()

        # ---------------- AllReduce of BN partial sums ----------------------
        if level >= 4:
            nc.vector.reduce_sum(
                _rap(sloc[:], [[2, 4], [1, 2]]),
                _rap(stats_cols[:], [[2, 4], [1, 2], [8, NS]]),
                axis=mybir.AxisListType.X)
            cc_pool = tc.alloc_tile_pool(name="ccd", bufs=1, space="DRAM")
            cc_in = cc_pool.tile([128, 8], F32, tag="cc_in")
            cc_out = cc_pool.tile([128, 8], F32, tag="cc_out")
            nc.sync.dma_start(out=cc_in[:], in_=sloc[:])
            nc.gpsimd.collective_compute(
                "AllReduce", mybir.AluOpType.add,
                replica_groups=[list(range(n_cores))],
                ins=[cc_in.opt()], outs=[cc_out.opt()])
            nc.sync.dma_start(out=sglob[:], in_=cc_out[:])

        # ---------------- pass C: out = cm*x + sc*y + bb ---------------------
        if level >= 5:
            pc_t = tc.alloc_tile_pool(name="pc_t", bufs=2)
            pc_o = tc.alloc_tile_pool(name="pc_o", bufs=2)
            HR = R // 2                   # pass-C half-strip rows

            if level >= 4:
                # ------------ BN coefficients (per out-channel) -------------
                bn_ps = tc.alloc_tile_pool(name="bn_ps", bufs=1, space="PSUM")
                gps = bn_ps.tile([128, 8], F32, tag="gps")
                nc.tensor.matmul(gps[:], gsum[:], sglob[:], start=True,
                                 stop=True)
                nc.vector.tensor_copy(msb[:], gps[:])
                bn_ps.release()
                mean = scalars[:, 4:5]
                e2 = scalars[:, 5:6]
                msq = scalars[:, 6:7]
                var = scalars[:, 7:8]
                sd = scalars[:, 8:9]
                rstd = scalars[:, 9:10]
                sc = scalars[:, 10:11]
                bb0 = scalars[:, 11:12]
                bb = scalars[:, 12:13]
                nc.vector.tensor_mul(sel[:], _rap(msb[:], [[2, 4]]), mask4[:])
                nc.vector.reduce_sum(mean, sel[:], axis=mybir.AxisListType.X)
                nc.vector.tensor_scalar_mul(mean, mean, 1.0 / N_TOT)
                nc.vector.tensor_mul(sel[:], _rap(msb[:], [[2, 4]], 1),
                                     mask4[:])
                nc.vector.reduce_sum(e2, sel[:], axis=mybir.AxisListType.X)
                nc.vector.tensor_scalar_mul(e2, e2, 1.0 / N_TOT)
                nc.vector.tensor_mul(msq, mean, mean)
                nc.vector.tensor_tensor(out=var, in0=e2, in1=msq,
                                        op=mybir.AluOpType.subtract)
                eps_ap = scalars[:, 13:14]
                nc.vector.memset(eps_ap, EPS)
                nc.scalar.activation(sd, var,
                                     mybir.ActivationFunctionType.Sqrt,
                                     bias=eps_ap)
                nc.vector.reciprocal(rstd, sd)
                nc.vector.tensor_scalar_mul(sc, rstd, NORM_SCALE)
                nc.vector.tensor_mul(bb0, mean, sc)
                nc.vector.tensor_scalar_mul(bb, bb0, -1.0)

            for h in range(2 * NS):
                st, half = h // 2, h % 2
                av_s = avls[st]
                if half == 1:
                    avls.pop(st)
                xsb = xsbs[st]
                if half == 1:
                    xsbs.pop(st)
                tv = pc_t.tile([128, HR * W], F32, tag="tv")
                HW2 = HR * W // 2
                # split the affine across scalar (front half) and vector
                # (back half), then the residual add on the opposite engine
                nc.scalar.activation(
                    _rap(tv[:], [[1, HW2]]),
                    _rap(av_s[:], [[1, HW2]], half * HR * W),
                    mybir.ActivationFunctionType.Identity,
                    bias=bb, scale=sc)
                nc.vector.tensor_scalar(
                    out=_rap(tv[:], [[1, HW2]], HW2),
                    in0=_rap(av_s[:], [[1, HW2]], half * HR * W + HW2),
                    scalar1=sc, scalar2=bb,
                    op0=mybir.AluOpType.mult, op1=mybir.AluOpType.add)
                o_s = pc_o.tile([128, HR * W], F32, tag="o_s")
                xoff = XW + 1 + half * HR * XW
                nc.vector.scalar_tensor_tensor(
                    out=_rap(o_s[:], [[1, HW2]]),
                    in0=_rap(xsb[:], [[XW, HR // 2], [1, W]], xoff),
                    scalar=float(cm), in1=_rap(tv[:], [[1, HW2]]),
                    op0=mybir.AluOpType.mult, op1=mybir.AluOpType.add)
                xc2 = pc_t.tile([128, HW2], F32, tag="xc2")
                nc.scalar.activation(
                    xc2[:],
                    _rap(xsb[:], [[XW, HR // 2], [1, W]],
                         xoff + (HR // 2) * XW),
                    mybir.ActivationFunctionType.Identity, scale=float(cm))
                nc.gpsimd.tensor_add(
                    _rap(o_s[:], [[1, HW2]], HW2), xc2[:],
                    _rap(tv[:], [[1, HW2]], HW2))
                nc.sync.dma_start(out=out_d[:, h * HR:(h + 1) * HR, :],
                                  in_=o_s[:])
            pc_o.release(); pc_t.release(); pc_a.release()

        if level >= 4:
            cc_pool.release()
        pa_xb.release()
        small.release()
        consts.release()

    nc.compile()
    return nc


def _av_load(nc, pool, avp_d, st, R, W):
    av_s = pool.tile([128, R * W], FP8, tag="av_s")
    nc.gpsimd.dma_start(out=av_s[:], in_=avp_d[:, st * R:(st + 1) * R, :])
    return av_s


def _prep_wt(w, dt, permute_out=False):
    """[Co,Ci,3,3] -> lhsT layout [Ci, 9, Co] (optionally out-chan permuted)."""
    wt = np.ascontiguousarray(w.transpose(1, 2, 3, 0).reshape(128, 9, 128))
    if permute_out:
        p = np.arange(128)
        co_of_p = 4 * (p % 32) + p // 32     # partition p holds channel co_of_p
        wt = np.ascontiguousarray(wt[:, :, co_of_p])
    return np.ascontiguousarray(wt.astype(dt))


def make_const_inputs(w1, w2, w3):
    import ml_dtypes
    E4 = ml_dtypes.float8_e4m3
    BF = ml_dtypes.bfloat16
    ident_bf = np.eye(128, dtype=np.float32).astype(BF)
    p = np.arange(128)
    # gsum[p_src, C']: sum av partitions with p_src%32 == C'//4
    gsum = (p[:, None] % 32 == p[None, :] // 4).astype(np.float32)
    mask4 = (p[:, None] % 4 == np.arange(4)[None, :]).astype(np.float32)
    return {
        "w1t": _prep_wt(np.asarray(w1, np.float32), BF),
        "w2t": _prep_wt(np.asarray(w2, np.float32), BF, permute_out=True),
        "w3t": _prep_wt(np.asarray(w3, np.float32), E4),
        "ident": ident_bf,
        "gsum": gsum,
        "mask4": mask4,
    }


def pad_x(x_sample):
    return np.pad(x_sample, ((0, 0), (1, 1), (1, 1)))


def make_in_maps(x, w1, w2, w3):
    import ml_dtypes
    consts = make_const_inputs(w1, w2, w3)
    in_maps = []
    for b in range(x.shape[0]):
        xp = pad_x(np.asarray(x[b], np.float32))
        m = dict(consts,
                 xb=np.ascontiguousarray(xp.astype(ml_dtypes.bfloat16)),
                 x8=np.ascontiguousarray(xp.astype(ml_dtypes.float8_e4m3)))
        in_maps.append(m)
    return in_maps


_CACHE = {}


def kernel(x, w1, w2, w3, conv_momentum):
    from concourse.bass_utils import run_bass_kernel_spmd

    x = np.asarray(x, np.float32)
    B, Ci, H, W = x.shape
    cm = float(np.asarray(conv_momentum))
    key = (H, W, B, cm)
    if key not in _CACHE:
        _CACHE[key] = build_nc(H, W, 24, B, cm)
    nc = _CACHE[key]
    in_maps = make_in_maps(x, w1, w2, w3)
    res = run_bass_kernel_spmd(nc, in_maps, list(range(B)))
    out = np.stack(
        [np.asarray(res.results[b]["out"]).reshape(128, H, W) for b in range(B)],
        axis=0)
    return out.astype(np.float32)


# revision 26
# speedup vs baseline: 1.0370x; 1.0370x over previous
"""Trainium2 Bass kernel for nn_AttnConv2d (attention-conv + dynamic conv + BN).

Math (per sample b):
  a1 = conv3x3(x, w1); a2 = conv3x3(x, w2); a3 = conv3x3(x, w3)     (SAME pad)
  attn[h,w,i,o] = sum_{p,q} a1[i,3p+h,3q+w] * a2[o,3p+h,3q+w]
  kern[o,:,:,:] = softmax(attn[.,.,.,o] / sqrt(Ci*9))
  av = conv3x3(a3, kern[b])                                         (per-sample kernel)
  y  = feature_map_stack(av)   (pure spatial/channel permutation)
  out = cm * x + NORM_SCALE * (y - mean_y) * rsqrt(var_y + eps)     (batch stats)

Sharding: data-parallel over batch, 1 sample per core, 8 cores.  The only
cross-core exchange is an AllReduce of the per-channel BN partial sums.

Implementation notes:
  - x arrives host-padded ([128, H+2, W+2]) in bf16 (a1/a2 convs) and fp8
    (a3 conv), so convs are 9 shifted accumulating matmuls into PSUM with
    no on-device edge handling.  a3 and the dynamic conv run fp8 with
    DoubleRow perf mode (two kernel offsets per matmul, K=256); fp8 there
    costs ~1e-2 rel err (attention path must stay bf16: fp8 a1/a2 alone
    is 1.8e-2).
  - attention contraction needs positions on the partition axis: conv
    outputs are scatter-copied to subgrid-major SBUF (bf16), PE-transposed
    in 128-position chunks, then accumulated into a persistent PSUM tile.
  - a2's output-channel order is permuted host-side (partition p holds
    channel 4*(p%32) + p//32) so feature_map_stack becomes a
    per-(partition,parity) affine map on av.
  - feature_map_stack is applied strip-by-strip during pass B as
    SBUF->SBUF scatter DMAs into a y-layout tile, so the permutation's
    descriptor cost overlaps the dynamic-conv compute; pass C then runs
    entirely on clean contiguous DMAs (x strip load, out strip store).
  - BN group-of-4 partition sums are a tiny 0/1 matmul; x prefetch is
    issued before the AllReduce so the collective latency is hidden.
  - strip pipeline is skewed: transposes+attention for strip s issue after
    the convs of strip s+1, so the scalar-engine scatter never stalls PE.
"""

import os
import sys

for _p in ("/opt/trn_rl_repo", "/root/.axon_site/_ro/trn_rl_repo"):
    if os.path.isdir(_p) and _p not in sys.path:
        sys.path.insert(0, _p)
        break

import numpy as np

import concourse.bass as bass
import concourse.bacc as bacc
import concourse.tile as tile
from concourse import mybir

F32 = mybir.dt.float32
BF16 = mybir.dt.bfloat16
FP8 = mybir.dt.float8e4
DR = mybir.MatmulPerfMode.DoubleRow

EPS = 1e-5
NORM_SCALE = 0.1816
CI = 128

# a3/dynamic-conv path runs fp8 (attention path stays bf16); DR_CONV
# selects DoubleRow pairing (2 offsets per matmul) vs plain fp8 matmuls.
DR_CONV = os.environ.get("DR_CONV", "1") == "1"


def _rap(base, dims, off=0):
    """Raw AP on the same tensor as `base` (keeps base's partition dim)."""
    return bass.AP(tensor=base.tensor, offset=base.offset + off,
                   ap=[base.ap[0]] + [list(d) for d in dims])


def _conv_tile(nc, cps, wt, xs, base, W, XW, dr):
    """Accumulate the 9-offset conv into PSUM cps [128, 2*W] for the row
    pair whose top-left (unshifted) element is at linear offset `base`
    within xs (a padded [128, rows, XW] tile)."""
    if not dr:
        for k in range(9):
            dy, dx = divmod(k, 3)
            rhs = _rap(xs[:], [[XW, 2], [1, W]], base + dy * XW + dx)
            nc.tensor.matmul(cps[:, :], wt[:, k, :], rhs,
                             start=(k == 0), stop=(k == 8))
        return
    deltas = [dy * XW + dx for dy in range(3) for dx in range(3)]
    for j in range(4):
        da, db = deltas[2 * j], deltas[2 * j + 1]
        rhs = _rap(xs[:], [[db - da, 2], [XW, 2], [1, W]], base + da)
        nc.tensor.matmul(cps[:, :], wt[:, 2 * j:2 * j + 2, :], rhs,
                         start=(j == 0), stop=False, perf_mode=DR,
                         skip_group_check=True)
    rhs = _rap(xs[:], [[XW, 2], [1, W]], base + deltas[8])
    nc.tensor.matmul(cps[:, :], wt[:, 8, :], rhs,
                     start=False, stop=True, skip_group_check=True)


def build_nc(H, W, R, n_cores, cm, level=5):
    """Build the per-core Bass kernel. R = strip rows (div by 6, even)."""
    assert H % R == 0 and R % 6 == 0 and W % 6 == 0
    NS = H // R                      # strips
    Wq = W // 3                      # attn subgrid cols
    P = (R // 3) * Wq                # attn positions per offset per strip
    S = H // 2                       # quadrant size of feature_map_stack
    NT = R // 2                      # psum tiles (2 rows) per strip
    SR = R // 2                      # subgrid rows per strip (parity space)
    SQ = W // 2                      # subgrid cols (parity space)
    N_TOT = float(n_cores * H * W)   # BN count per channel
    SCL = 1.0 / float(np.sqrt(CI * 9))
    XW = W + 2                       # padded row pitch

    nc = bacc.Bacc("TRN2", target_bir_lowering=False, debug=False,
                   num_devices=n_cores)

    xb_in = nc.dram_tensor("xb", [128, H + 2, XW], BF16,
                           kind="ExternalInput").ap()
    x8_in = nc.dram_tensor("x8", [128, H + 2, XW], FP8,
                           kind="ExternalInput").ap()
    w1_in = nc.dram_tensor("w1t", [128, 9, 128], BF16,
                           kind="ExternalInput").ap()
    w2_in = nc.dram_tensor("w2t", [128, 9, 128], BF16,
                           kind="ExternalInput").ap()
    w3_in = nc.dram_tensor("w3t", [128, 9, 128], FP8,
                           kind="ExternalInput").ap()
    id_in = nc.dram_tensor("ident", [128, 128], BF16, kind="ExternalInput").ap()
    gp_in = nc.dram_tensor("gsum", [128, 128], F32, kind="ExternalInput").ap()
    mk_in = nc.dram_tensor("mask4", [128, 4], F32, kind="ExternalInput").ap()
    out_d = nc.dram_tensor("out", [128, H, W], F32, kind="ExternalOutput").ap()
    avp_d = nc.dram_tensor("avp", [128, H, W], FP8).ap()   # scratch, y layout

    with tile.TileContext(nc) as tc:
        consts = tc.alloc_tile_pool(name="consts", bufs=1)
        w1t = consts.tile([128, 9, 128], BF16, tag="w1t")
        w2t = consts.tile([128, 9, 128], BF16, tag="w2t")
        w3t = consts.tile([128, 9, 128], FP8, tag="w3t")
        ident = consts.tile([128, 128], BF16, tag="ident")
        gsum = consts.tile([128, 128], F32, tag="gsum")
        mask4 = consts.tile([128, 4], F32, tag="mask4")
        nc.sync.dma_start(out=w1t[:], in_=w1_in[:])
        nc.sync.dma_start(out=w2t[:], in_=w2_in[:])
        nc.sync.dma_start(out=w3t[:], in_=w3_in[:])
        nc.sync.dma_start(out=ident[:], in_=id_in[:])
        nc.sync.dma_start(out=gsum[:], in_=gp_in[:])
        nc.sync.dma_start(out=mask4[:], in_=mk_in[:])

        small = tc.alloc_tile_pool(name="small", bufs=1)
        stats_cols = small.tile([128, NS, 4, 2], F32, tag="stats_cols")
        sloc = small.tile([128, 8], F32, tag="sloc")
        sglob = small.tile([128, 8], F32, tag="sglob")
        scalars = small.tile([128, 16], F32, tag="scalars")
        msb = small.tile([128, 8], F32, tag="msb")
        sel = small.tile([128, 4], F32, tag="sel")

        # xb strips persist from pass A through pass C (residual input)
        pa_xb = tc.alloc_tile_pool(name="pa_xb", bufs=NS)
        # pass-C av strip tiles (prefetched during pass B)
        pc_a = tc.alloc_tile_pool(name="pc_a", bufs=8)

        kern_pool = tc.alloc_tile_pool(name="kern", bufs=1)
        kern8 = kern_pool.tile([128, 9, 128], FP8, tag="kern8")

        a3_pool = tc.alloc_tile_pool(name="a3p", bufs=1)
        a3p = a3_pool.tile([128, H + 2, XW], FP8, tag="a3p")
        # zero the pad border of a3p once
        nc.vector.memset(_rap(a3p[:], [[1, XW]]), 0.0)                    # row 0
        nc.vector.memset(_rap(a3p[:], [[1, XW]], (H + 1) * XW), 0.0)      # row H+1
        nc.vector.memset(_rap(a3p[:], [[XW, H + 2]]), 0.0)                # col 0
        nc.vector.memset(_rap(a3p[:], [[XW, H + 2]], W + 1), 0.0)         # col W+1

        attn_psp = tc.alloc_tile_pool(name="attn_ps", bufs=1, space="PSUM")
        attn_ps = attn_psp.tile([128, 9 * 128], F32, tag="attn")

        # ---------------- pass A: static convs + attention accumulation ------
        pa_x = tc.alloc_tile_pool(name="pa_x", bufs=2)
        pa_g = tc.alloc_tile_pool(name="pa_g", bufs=2)
        pa_t = tc.alloc_tile_pool(name="pa_t", bufs=2)
        pa_cps = tc.alloc_tile_pool(name="pa_cps", bufs=3, space="PSUM")
        pa_tps = tc.alloc_tile_pool(name="pa_tps", bufs=2, space="PSUM")

        a1gs = {}
        a2gs = {}
        xsbs = {}

        def conv_strip(s):
            y0 = s * R
            xsb = pa_xb.tile([128, R + 2, XW], BF16, tag="xsb")
            xsbs[s] = xsb
            nc.sync.dma_start(out=xsb[:], in_=xb_in[:, y0:y0 + R + 2, :])
            xs8 = pa_x.tile([128, R + 2, XW], FP8, tag="xs8")
            nc.sync.dma_start(out=xs8[:], in_=x8_in[:, y0:y0 + R + 2, :])
            a1g = pa_g.tile([128, 9, P], BF16, tag="a1g")
            a2g = pa_g.tile([128, 9, P], BF16, tag="a2g")
            a1gs[s] = a1g
            a2gs[s] = a2g
            for wt, gdst, xt, fp8 in ((w1t, a1g, xsb, False),
                                      (w2t, a2g, xsb, False),
                                      (w3t, None, xs8, DR_CONV)):
                for t in range(NT):
                    cps = pa_cps.tile([128, 2 * W], F32, tag="cps")
                    _conv_tile(nc, cps, wt, xt, 2 * t * XW, W, XW, fp8)
                    if gdst is not None:
                        # scatter rows (2t, 2t+1) into subgrid-major layout
                        ya, yb = 2 * t, 2 * t + 1
                        ha, ra = ya % 3, ya // 3
                        hb, rb = yb % 3, yb // 3
                        offa = (3 * ha) * P + ra * Wq
                        sd = (3 * hb) * P + rb * Wq - offa
                        nc.scalar.copy(
                            out=_rap(gdst[:], [[sd, 2], [P, 3], [1, Wq]],
                                     offa),
                            in_=_rap(cps[:], [[W, 2], [1, 3], [3, Wq]]))
                    else:
                        nc.scalar.copy(
                            out=a3p[:, 1 + y0 + 2 * t:1 + y0 + 2 * t + 2,
                                    1:1 + W],
                            in_=_rap(cps[:], [[W, 2], [1, W]]))

        def attn_strip(s):
            a1g, a2g = a1gs.pop(s), a2gs.pop(s)
            for hw in range(9):
                for c0 in range(0, P, 256):
                    t4 = pa_tps.tile([128, 4, 128], BF16, tag="tps")
                    for j, (g, cc) in enumerate(((a1g, c0), (a2g, c0),
                                                 (a1g, c0 + 128),
                                                 (a2g, c0 + 128))):
                        nc.tensor.transpose(t4[:, j, :], g[:, hw, cc:cc + 128],
                                            ident[:])
                    aT4 = pa_t.tile([128, 4, 128], BF16, tag="aT")
                    nc.vector.tensor_copy(aT4[:], t4[:])
                    for j in range(2):
                        nc.tensor.matmul(
                            attn_ps[:, hw * 128:(hw + 1) * 128],
                            aT4[:, 2 * j + 1, :], aT4[:, 2 * j, :],
                            start=(s == 0 and c0 == 0 and j == 0
                                   and hw in (0, 4, 8)),
                            stop=(s == NS - 1 and c0 + 256 >= P and j == 1
                                  and hw in (3, 7, 8)),
                            skip_group_check=True)

        # skewed pipeline: the scalar/vector scatter of strip s drains while
        # PE runs the convs of strip s+1.
        for s in range(NS):
            conv_strip(s)
            if s > 0:
                attn_strip(s - 1)
        attn_strip(NS - 1)

        pa_tps.release(); pa_cps.release()
        pa_t.release(); pa_g.release(); pa_x.release()
        # pa_xb stays live: pass C reuses the xb strips for the residual

        # ---------------- softmax + kern transposes -------------------------
        if level >= 2:
            sm_pool = tc.alloc_tile_pool(name="smx", bufs=1)
            attn_sb = sm_pool.tile([128, 9 * 128], F32, tag="attn_sb")
            nc.vector.tensor_copy(attn_sb[:], attn_ps[:])
            attn_psp.release()
            k_tps = tc.alloc_tile_pool(name="k_tps", bufs=2, space="PSUM")
            mx = scalars[:, 0:1]
            nmx = scalars[:, 1:2]
            ssum = scalars[:, 2:3]
            rsum = scalars[:, 3:4]
            nc.vector.reduce_max(mx, attn_sb[:], axis=mybir.AxisListType.X)
            nc.vector.tensor_scalar_mul(nmx, mx, -SCL)
            esb = sm_pool.tile([128, 9 * 128], F32, tag="esb")
            nc.scalar.activation(esb[:], attn_sb[:],
                                 mybir.ActivationFunctionType.Exp,
                                 bias=nmx, scale=SCL)
            nc.vector.reduce_sum(ssum, esb[:], axis=mybir.AxisListType.X)
            nc.vector.reciprocal(rsum, ssum)
            sm_bf = sm_pool.tile([128, 9 * 128], BF16, tag="sm_bf")
            nc.vector.tensor_scalar_mul(sm_bf[:], esb[:], rsum)
            for hw in range(9):
                tp = k_tps.tile([128, 128], BF16, tag="ktp")
                nc.tensor.transpose(tp[:], sm_bf[:, hw * 128:(hw + 1) * 128],
                                    ident[:])
                nc.vector.tensor_copy(kern8[:, hw, :], tp[:])
            k_tps.release(); sm_pool.release()
        else:
            attn_psp.release()

        # ---------------- pass B: dynamic conv + stats + y scatter ----------
        if level >= 3:
            pb_av = tc.alloc_tile_pool(name="pb_av", bufs=4)
            pb_sq = tc.alloc_tile_pool(name="pb_sq", bufs=2)
            pb_cps = tc.alloc_tile_pool(name="pb_cps", bufs=5, space="PSUM")
            avs = {}

            def dyn_strip(s):
                y0 = s * R
                # av parity-split strip: av_sp[c, 2i+j, p, q] = av[c, 2p+i, 2q+j]
                av_sp = pb_av.tile([128, 4, SR, SQ], FP8, tag="av_sp")
                avs[s] = av_sp
                for t in range(NT):
                    cps = pb_cps.tile([128, 2 * W], F32, tag="cps2")
                    _conv_tile(nc, cps, kern8, a3p, (y0 + 2 * t) * XW, W, XW,
                               DR_CONV)
                    nc.scalar.copy(
                        out=_rap(av_sp[:], [[2 * SR * SQ, 2], [SR * SQ, 2],
                                            [1, SQ]], t * SQ),
                        in_=_rap(cps[:], [[W, 2], [1, 2], [2, SQ]]))

            def stats_strip(s):
                av_sp = avs.pop(s)
                sq = pb_sq.tile([128, SR * SQ], BF16, tag="sq")
                for pi in range(4):
                    psrc = _rap(av_sp[:], [[1, SR * SQ]], pi * SR * SQ)
                    nc.vector.reduce_sum(stats_cols[:, s, pi, 0:1], psrc,
                                         axis=mybir.AxisListType.X)
                    nc.vector.scalar_tensor_tensor(
                        out=sq[:], in0=psrc, scalar=1.0, in1=psrc,
                        op0=mybir.AluOpType.mult, op1=mybir.AluOpType.mult,
                        accum_out=stats_cols[:, s, pi, 1:2])
                # feature_map_stack fold: av[32*c2+c1, par, p, q]
                #   -> avp[4*c1+par, S*(c2>>1)+s*SR+p, S*(c2&1)+q]
                for c2 in range(4):
                    qsrc = _rap(av_sp[32 * c2:32 * (c2 + 1)],
                                [[SR * SQ, 4], [SQ, SR], [1, SQ]])
                    dst = bass.AP(
                        tensor=avp_d.tensor,
                        offset=((c2 >> 1) * S * W + (c2 & 1) * SQ
                                + (s * SR) * W),
                        ap=[[4 * H * W, 32], [H * W, 4], [W, SR], [1, SQ]])
                    nc.gpsimd.dma_start(out=dst, in_=qsrc)

            avls = {}
            nxt = [0]

            def prefetch_ready(smax):
                # avp out-strip st (rows st*R..) is fully written once av
                # strips {2*st, 2*st+1} (st < NS//2: upper half) or
                # {2*st-NS, 2*st-NS+1} (lower half) have scattered.
                while nxt[0] < NS:
                    st = nxt[0]
                    need = 2 * st + 1 if st < NS // 2 else 2 * st - NS + 1
                    if need > smax:
                        break
                    avls[st] = _av_load(nc, pc_a, avp_d, st, R, W)
                    nxt[0] += 1

            for s in range(NS):
                dyn_strip(s)
                if s > 0:
                    stats_strip(s - 1)
                    prefetch_ready(s - 1)
            stats_strip(NS - 1)
            prefetch_ready(NS - 1)
            pb_cps.release()
            pb_sq.release()
            pb_av.release()
        a3_pool.release(); kern_pool.release()

        # ---------------- AllReduce of BN partial sums ----------------------
        if level >= 4:
            nc.vector.reduce_sum(
                _rap(sloc[:], [[2, 4], [1, 2]]),
                _rap(stats_cols[:], [[2, 4], [1, 2], [8, NS]]),
                axis=mybir.AxisListType.X)
            cc_pool = tc.alloc_tile_pool(name="ccd", bufs=1, space="DRAM")
            cc_in = cc_pool.tile([128, 8], F32, tag="cc_in")
            cc_out = cc_pool.tile([128, 8], F32, tag="cc_out")
            nc.sync.dma_start(out=cc_in[:], in_=sloc[:])
            nc.gpsimd.collective_compute(
                "AllReduce", mybir.AluOpType.add,
                replica_groups=[list(range(n_cores))],
                ins=[cc_in.opt()], outs=[cc_out.opt()])
            nc.sync.dma_start(out=sglob[:], in_=cc_out[:])

        # ---------------- pass C: out = cm*x + sc*y + bb ---------------------
        if level >= 5:
            pc_t = tc.alloc_tile_pool(name="pc_t", bufs=2)
            pc_o = tc.alloc_tile_pool(name="pc_o", bufs=2)
            HR = R // 2                   # pass-C half-strip rows

            if level >= 4:
                # ------------ BN coefficients (per out-channel) -------------
                bn_ps = tc.alloc_tile_pool(name="bn_ps", bufs=1, space="PSUM")
                gps = bn_ps.tile([128, 8], F32, tag="gps")
                nc.tensor.matmul(gps[:], gsum[:], sglob[:], start=True,
                                 stop=True)
                nc.vector.tensor_copy(msb[:], gps[:])
                bn_ps.release()
                mean = scalars[:, 4:5]
                e2 = scalars[:, 5:6]
                msq = scalars[:, 6:7]
                var = scalars[:, 7:8]
                sd = scalars[:, 8:9]
                rstd = scalars[:, 9:10]
                sc = scalars[:, 10:11]
                bb0 = scalars[:, 11:12]
                bb = scalars[:, 12:13]
                nc.vector.tensor_mul(sel[:], _rap(msb[:], [[2, 4]]), mask4[:])
                nc.vector.reduce_sum(mean, sel[:], axis=mybir.AxisListType.X)
                nc.vector.tensor_scalar_mul(mean, mean, 1.0 / N_TOT)
                nc.vector.tensor_mul(sel[:], _rap(msb[:], [[2, 4]], 1),
                                     mask4[:])
                nc.vector.reduce_sum(e2, sel[:], axis=mybir.AxisListType.X)
                nc.vector.tensor_scalar_mul(e2, e2, 1.0 / N_TOT)
                nc.vector.tensor_mul(msq, mean, mean)
                nc.vector.tensor_tensor(out=var, in0=e2, in1=msq,
                                        op=mybir.AluOpType.subtract)
                eps_ap = scalars[:, 13:14]
                nc.vector.memset(eps_ap, EPS)
                nc.scalar.activation(sd, var,
                                     mybir.ActivationFunctionType.Sqrt,
                                     bias=eps_ap)
                nc.vector.reciprocal(rstd, sd)
                nc.vector.tensor_scalar_mul(sc, rstd, NORM_SCALE)
                nc.vector.tensor_mul(bb0, mean, sc)
                nc.vector.tensor_scalar_mul(bb, bb0, -1.0)

            for h in range(2 * NS):
                st, half = h // 2, h % 2
                av_s = avls[st]
                if half == 1:
                    avls.pop(st)
                xsb = xsbs[st]
                if half == 1:
                    xsbs.pop(st)
                tv = pc_t.tile([128, HR * W], F32, tag="tv")
                nc.scalar.activation(
                    tv[:], _rap(av_s[:], [[1, HR * W]], half * HR * W),
                    mybir.ActivationFunctionType.Identity,
                    bias=bb, scale=sc)
                o_s = pc_o.tile([128, HR * W], F32, tag="o_s")
                xoff = XW + 1 + half * HR * XW
                nc.vector.scalar_tensor_tensor(
                    out=o_s[:], in0=_rap(xsb[:], [[XW, HR], [1, W]], xoff),
                    scalar=float(cm), in1=tv[:],
                    op0=mybir.AluOpType.mult, op1=mybir.AluOpType.add)
                nc.sync.dma_start(out=out_d[:, h * HR:(h + 1) * HR, :],
                                  in_=o_s[:])
            pc_o.release(); pc_t.release(); pc_a.release()

        if level >= 4:
            cc_pool.release()
        pa_xb.release()
        small.release()
        consts.release()

    nc.compile()
    return nc


def _av_load(nc, pool, avp_d, st, R, W):
    av_s = pool.tile([128, R * W], FP8, tag="av_s")
    nc.gpsimd.dma_start(out=av_s[:], in_=avp_d[:, st * R:(st + 1) * R, :])
    return av_s


def _prep_wt(w, dt, permute_out=False):
    """[Co,Ci,3,3] -> lhsT layout [Ci, 9, Co] (optionally out-chan permuted)."""
    wt = np.ascontiguousarray(w.transpose(1, 2, 3, 0).reshape(128, 9, 128))
    if permute_out:
        p = np.arange(128)
        co_of_p = 4 * (p % 32) + p // 32     # partition p holds channel co_of_p
        wt = np.ascontiguousarray(wt[:, :, co_of_p])
    return np.ascontiguousarray(wt.astype(dt))


def make_const_inputs(w1, w2, w3):
    import ml_dtypes
    E4 = ml_dtypes.float8_e4m3
    BF = ml_dtypes.bfloat16
    ident_bf = np.eye(128, dtype=np.float32).astype(BF)
    p = np.arange(128)
    # gsum[p_src, C']: sum av partitions with p_src%32 == C'//4
    gsum = (p[:, None] % 32 == p[None, :] // 4).astype(np.float32)
    mask4 = (p[:, None] % 4 == np.arange(4)[None, :]).astype(np.float32)
    return {
        "w1t": _prep_wt(np.asarray(w1, np.float32), BF),
        "w2t": _prep_wt(np.asarray(w2, np.float32), BF, permute_out=True),
        "w3t": _prep_wt(np.asarray(w3, np.float32), E4),
        "ident": ident_bf,
        "gsum": gsum,
        "mask4": mask4,
    }


def pad_x(x_sample):
    return np.pad(x_sample, ((0, 0), (1, 1), (1, 1)))


def make_in_maps(x, w1, w2, w3):
    import ml_dtypes
    consts = make_const_inputs(w1, w2, w3)
    in_maps = []
    for b in range(x.shape[0]):
        xp = pad_x(np.asarray(x[b], np.float32))
        m = dict(consts,
                 xb=np.ascontiguousarray(xp.astype(ml_dtypes.bfloat16)),
                 x8=np.ascontiguousarray(xp.astype(ml_dtypes.float8_e4m3)))
        in_maps.append(m)
    return in_maps


_CACHE = {}


def kernel(x, w1, w2, w3, conv_momentum):
    from concourse.bass_utils import run_bass_kernel_spmd

    x = np.asarray(x, np.float32)
    B, Ci, H, W = x.shape
    cm = float(np.asarray(conv_momentum))
    key = (H, W, B, cm)
    if key not in _CACHE:
        _CACHE[key] = build_nc(H, W, 24, B, cm)
    nc = _CACHE[key]
    in_maps = make_in_maps(x, w1, w2, w3)
    res = run_bass_kernel_spmd(nc, in_maps, list(range(B)))
    out = np.stack(
        [np.asarray(res.results[b]["out"]).reshape(128, H, W) for b in range(B)],
        axis=0)
    return out.astype(np.float32)
